# revision 1
# baseline (speedup 1.0000x reference)
"""Trainium2 Bass kernel for nn_BuildCombinationsDim2 (k=2 feature-pair gather).

Reference computation: x [B=32, T=4096, F=32] f32, k=2 ->
out[..., j] = x[..., idx[j]] where idx is the flattened list of all
C(32,2)=496 lexicographic feature pairs -> out [32, 4096, 992].

Strategy (pure data movement, memory-bound on the 520MB output write):
  - Shard batch across 8 cores: each core gets [4, 4096, 32] -> 16384 rows.
  - Per core, tile rows onto 128 SBUF partitions, R=8 rows per partition.
  - The 992 output columns decompose into 31 blocks; block i (pairs
    (i,i+1)..(i,31)) has its even slots all equal to x[:,i] (a stride-0
    broadcast copy) and its odd slots equal to the contiguous suffix
    x[:,i+1:32] (a stride-2-dest copy). 62 strided on-chip copies build a
    full [128, R*992] tile, stored as one 4MB contiguous HBM write.
  - Input loads go on the ACT HWDGE ring (nc.scalar.dma_start). They MUST
    NOT use nc.gpsimd (SWDGE): fp32 DVE tensor_copy runs in 2-port perf
    mode, which locks GPSIMD out of the shared SBUF port pair, starving
    SWDGE descriptor generation and serializing loads behind every DVE op
    (~+120us/body measured). HWDGE never contends with engines.
  - Each tile's 31 blocks are split between DVE (even i) and ACT (odd i)
    so both engines build every tile concurrently; output stores stream
    on the sync (SP) HWDGE ring, which is the ~358 GB/s HBM-per-NC
    bottleneck (~182us/core steady state for the 65MB/core write).
"""

import numpy as np

import concourse.bacc as bacc
import concourse.bass as bass
import concourse.mybir as mybir
from concourse.bass_utils import run_bass_kernel_spmd
from concourse.tile import TileContext

F = 32
NCR = F * (F - 1)  # 992 = 2 * C(32,2)
N_CORES = 8
P = 128

B_FULL, T_FULL = 32, 4096
ROWS_PER_CORE = (B_FULL // N_CORES) * T_FULL  # 16384

R_DEFAULT = 8        # rows per partition per tile
BUFS_DEFAULT = 4     # output-tile buffering
IN_DMA_DEFAULT = "scalar"  # input DMA on the ACT HWDGE ring (NOT SWDGE)
IN_BUFS_DEFAULT = 8  # input prefetch depth (per-tile load mode only)
ENGINES_DEFAULT = ("vector",)  # DVE-only tile builds (8.4us/tile < 11.6us
                               # store cadence; single-sem dep per store)
PRELOAD_DEFAULT = True  # one 2MB input DMA per body: 16 interleaved
                        # per-tile loads cost ~16us/body of store throughput
                        # (HBM read/write turnarounds in the write stream)
RAMP_DEFAULT = (2, 2, 2, 2)  # four 2-row opener tiles fed from a separate
                             # 128KB head-preload tile: first store issues
                             # after ~3us instead of ~14us


def build_nc(rows=ROWS_PER_CORE, r_per_part=R_DEFAULT, bufs=BUFS_DEFAULT,
             engines=ENGINES_DEFAULT, repeat=1, in_dma=IN_DMA_DEFAULT,
             in_bufs=IN_BUFS_DEFAULT, out_dma="sync", loop=0,
             preload=PRELOAD_DEFAULT, unroll=1,
             split_pat=("vector", "scalar"), group=1, ramp=None):
    """Build the per-core Bass module: x [rows, 32] -> out [rows, 992].

    engines: "split" assigns block i of every tile to DVE (i even) or ACT
    (i odd); a tuple like ("vector", "scalar") instead alternates whole
    tiles between engines. repeat>1 unrolls the whole body; loop=N wraps
    the body in a tc.For_i hardware loop (timing harness use).
    """
    ramp = RAMP_DEFAULT if ramp is None else ramp
    if not preload or group > 1:
        ramp = ()
    tile_rows = P * r_per_part
    assert rows % tile_rows == 0
    n_tiles = rows // tile_rows
    R = r_per_part

    nc = bacc.Bacc(
        "TRN2", target_bir_lowering=False, debug=False, num_devices=N_CORES
    )
    x = nc.dram_tensor("x", [rows, F], mybir.dt.float32, kind="ExternalInput")
    out = nc.dram_tensor("out", [rows, NCR], mybir.dt.float32,
                         kind="ExternalOutput")

    in_eng = {"sync": nc.sync, "scalar": nc.scalar, "gpsimd": nc.gpsimd}[in_dma]
    def out_eng_for(t):
        if out_dma == "alt":
            return nc.sync if t % 2 == 0 else nc.scalar
        return {"sync": nc.sync, "scalar": nc.scalar}[out_dma]

    if preload:
        J = rows // P
        x_v = x.rearrange("(p j) c -> p (j c)", p=P)      # [128, J*F]
        out_v = out.rearrange("(p j) c -> p (j c)", p=P)  # [128, J*NCR]
    else:
        x_t = x.rearrange("(t p r) c -> t p (r c)", p=P, r=R)
        out_t = out.rearrange("(t p r) c -> t p (r c)", p=P, r=R)

    def build_tile(t, xt, ot, rr=None):
        rr = R if rr is None else rr
        x3 = xt.rearrange("p (r c) -> p r c", r=rr)
        o3 = ot.rearrange("p (r c) -> p r c", r=rr)
        col = 0
        for i in range(F - 1):
            w = F - 1 - i  # number of pairs starting with feature i
            dst_even = o3[:, :, col:col + 2 * w:2]
            dst_odd = o3[:, :, col + 1:col + 2 * w:2]
            src_b = x3[:, :, i:i + 1].broadcast_to([P, rr, w])
            src_s = x3[:, :, i + 1:F]
            if engines == "split":
                eng = split_pat[i % len(split_pat)]
            else:
                eng = engines[t % len(engines)]
            if eng == "vector":
                nc.vector.tensor_copy(dst_even, src_b)
                nc.vector.tensor_copy(dst_odd, src_s)
            else:
                nc.scalar.copy(dst_even, src_b)
                nc.scalar.copy(dst_odd, src_s)
            col += 2 * w

    def body_ramp(pool, xa, xb):
        # xa holds the first sum(ramp) rows, xb the rest: tile-0 copies
        # only wait on the small head load, so the first store issues
        # ~10us earlier than with one monolithic preload.
        head = sum(ramp)
        in_eng.dma_start(xa[:, :], x_v[:, :head * F])
        in_eng.dma_start(xb[:, :], x_v[:, head * F:])
        j = 0
        for si, rr in enumerate(list(ramp) + [R] * ((rows // P - head) // R)):
            if j < head:
                xt = xa[:, j * F:(j + rr) * F]
            else:
                xt = xb[:, (j - head) * F:(j - head + rr) * F]
            ot = pool.tile([P, R * NCR], mybir.dt.float32, name="ot")
            build_tile(si, xt, ot[:, :rr * NCR], rr=rr)
            out_eng_for(si).dma_start(
                out_v[:, j * NCR:(j + rr) * NCR], ot[:, :rr * NCR])
            j += rr

    def body(pool, xall=None):
        if group > 1:
            # partition-major preload layout: `group` consecutive tiles are
            # contiguous per partition -> one store, half the DMA boundaries
            assert preload and n_tiles % group == 0
            for b in range(n_tiles // group):
                ob = pool.tile([P, group * R * NCR], mybir.dt.float32,
                               name="ob")
                for g in range(group):
                    t = b * group + g
                    xt = xall[:, t * R * F:(t + 1) * R * F]
                    build_tile(t, xt,
                               ob[:, g * R * NCR:(g + 1) * R * NCR])
                out_eng_for(b).dma_start(
                    out_v[:, b * group * R * NCR:
                          (b + 1) * group * R * NCR], ob[:, :])
            return
        for t in range(n_tiles):
            if preload:
                xt = xall[:, t * R * F:(t + 1) * R * F]
            else:
                xtile = pool.tile([P, R * F], mybir.dt.float32, name="xt",
                                  bufs=in_bufs)
                in_eng.dma_start(xtile[:, :], x_t[t])
                xt = xtile[:, :]
            ot = pool.tile([P, R * NCR], mybir.dt.float32, name="ot")
            build_tile(t, xt, ot[:, :])
            if preload:
                out_eng_for(t).dma_start(
                    out_v[:, t * R * NCR:(t + 1) * R * NCR], ot[:, :])
            else:
                out_eng_for(t).dma_start(out_t[t], ot[:, :])

    with TileContext(nc) as tc:
        with tc.tile_pool(name="pool", bufs=bufs) as pool:
            xall = xa = xb = None
            if ramp:
                assert preload and sum(ramp) % 2 == 0
                head = sum(ramp)
                xa = pool.tile([P, head * F], mybir.dt.float32,
                               name="xa", bufs=1)
                xb = pool.tile([P, (rows // P - head) * F],
                               mybir.dt.float32, name="xb", bufs=1)
            elif preload:
                xall = pool.tile([P, (rows // P) * F], mybir.dt.float32,
                                 name="xall", bufs=1)
            if loop:
                with tc.For_i(0, loop, 1):
                    for _ in range(unroll):
                        if ramp:
                            body_ramp(pool, xa, xb)
                            continue
                        if preload:
                            in_eng.dma_start(xall[:, :], x_v)
                        body(pool, xall)
            else:
                for rep in range(repeat):
                    if ramp:
                        body_ramp(pool, xa, xb)
                        continue
                    if preload:
                        in_eng.dma_start(xall[:, :], x_v)
                    body(pool, xall)
    nc.finalize()
    return nc


_NC_CACHE = {}


def _get_nc():
    key = (ROWS_PER_CORE, R_DEFAULT, BUFS_DEFAULT, IN_DMA_DEFAULT,
           ENGINES_DEFAULT, PRELOAD_DEFAULT)
    if key not in _NC_CACHE:
        _NC_CACHE[key] = build_nc()
    return _NC_CACHE[key]


def kernel(x, k=2):
    x = np.ascontiguousarray(np.asarray(x), dtype=np.float32)
    assert int(np.asarray(k)) == 2, "kernel hardcodes k=2"
    B, T, Fin = x.shape
    assert (B, T, Fin) == (B_FULL, T_FULL, F)

    xf = x.reshape(N_CORES, ROWS_PER_CORE, F)
    in_maps = [{"x": xf[c]} for c in range(N_CORES)]
    nc = _get_nc()
    res = run_bass_kernel_spmd(nc, in_maps, list(range(N_CORES)))
    outs = [np.asarray(res.results[c]["out"]) for c in range(N_CORES)]
    return np.concatenate(outs, axis=0).reshape(B, T, NCR)



# revision 6
# speedup vs baseline: 1.0545x; 1.0545x over previous
"""Trainium2 Bass kernel for nn_BuildCombinationsDim2 (k=2 feature-pair gather).

Reference computation: x [B=32, T=4096, F=32] f32, k=2 ->
out[..., j] = x[..., idx[j]] where idx is the flattened list of all
C(32,2)=496 lexicographic feature pairs -> out [32, 4096, 992].

Strategy (pure data movement, memory-bound on the 520MB output write):
  - Shard batch across 8 cores: each core gets [4, 4096, 32] -> 16384 rows.
  - Per core, tile rows onto 128 SBUF partitions, R=8 rows per partition.
  - The 992 output columns decompose into 31 blocks; block i (pairs
    (i,i+1)..(i,31)) has its even slots all equal to x[:,i] (a stride-0
    broadcast copy) and its odd slots equal to the contiguous suffix
    x[:,i+1:32] (a stride-2-dest copy). 62 strided on-chip copies build a
    full [128, R*992] tile, stored as one 4MB contiguous HBM write.
  - Measured floor (this container): store stream ~347 GB/s -> 187us for
    the 62MB/core write; +2MB input load -> ~195us/body. Best measured
    kernel: ~197us/body.
  - fp32 tensor_copy with strided dst / broadcast src runs at 1x mode
    (~1.7 ns/elem/partition), NOT the dense 2x rate: a DVE-only tile build
    is ~13.3us, ABOVE the ~11.1us store cadence -> DVE-only is build-bound
    (~213us). engines="split" assigns block i to DVE (i even) or ACT
    (i odd): ~4us/tile per engine, builds come off the critical path.
  - Input loads go on the ACT HWDGE ring (nc.scalar.dma_start), NOT
    nc.gpsimd/SWDGE (GPSIMD descriptor gen starves behind DVE port locks,
    ~+120us/body) and NOT the sync ring (delays store descriptor gen,
    +10us/body measured at unroll=4).
  - tc.For_i iteration boundaries cost ~4-6us (cross-queue sync); timing
    NEFFs unroll 4 full bodies per iteration to amortize. out_dma="alt",
    group>1, r_per_part 4/16, bufs 2/5/6, prebufs=2 all measured
    neutral-to-worse than sync/R=8/bufs=4.
"""

import numpy as np

import concourse.bacc as bacc
import concourse.bass as bass
import concourse.mybir as mybir
from concourse.bass_utils import run_bass_kernel_spmd
from concourse.tile import TileContext

F = 32
NCR = F * (F - 1)  # 992 = 2 * C(32,2)
N_CORES = 8
P = 128

B_FULL, T_FULL = 32, 4096
ROWS_PER_CORE = (B_FULL // N_CORES) * T_FULL  # 16384

R_DEFAULT = 8        # rows per partition per tile
BUFS_DEFAULT = 4     # output-tile buffering
IN_DMA_DEFAULT = "scalar"  # input DMA on the ACT HWDGE ring (NOT SWDGE)
IN_BUFS_DEFAULT = 8  # input prefetch depth (per-tile load mode only)
ENGINES_DEFAULT = "split"  # co-build every tile on DVE (even blocks) + ACT
                           # (odd blocks); DVE-only builds are build-bound
PRELOAD_DEFAULT = True  # one 2MB input DMA per body: 16 interleaved
                        # per-tile loads cost ~16us/body of store throughput
                        # (HBM read/write turnarounds in the write stream)
RAMP_DEFAULT = (2, 2, 2, 2)  # four 2-row opener tiles fed from a separate
                             # 128KB head-preload tile: first store issues
                             # after ~3us instead of ~14us


def build_nc(rows=ROWS_PER_CORE, r_per_part=R_DEFAULT, bufs=BUFS_DEFAULT,
             engines=ENGINES_DEFAULT, repeat=1, in_dma=IN_DMA_DEFAULT,
             in_bufs=IN_BUFS_DEFAULT, out_dma="sync", loop=0,
             preload=PRELOAD_DEFAULT, unroll=1,
             split_pat=("vector", "scalar"), group=1, ramp=None,
             prebufs=1):
    """Build the per-core Bass module: x [rows, 32] -> out [rows, 992].

    engines: "split" assigns block i of every tile to DVE (i even) or ACT
    (i odd); a tuple like ("vector", "scalar") instead alternates whole
    tiles between engines. repeat>1 unrolls the whole body; loop=N wraps
    the body in a tc.For_i hardware loop (timing harness use).
    """
    if ramp is None:
        # ramp cuts single-shot cold-start; in loop (timing) mode it only
        # adds small-store overhead per body.
        ramp = RAMP_DEFAULT if loop == 0 else ()
    if not preload or group > 1:
        ramp = ()
    tile_rows = P * r_per_part
    assert rows % tile_rows == 0
    n_tiles = rows // tile_rows
    R = r_per_part

    nc = bacc.Bacc(
        "TRN2", target_bir_lowering=False, debug=False, num_devices=N_CORES
    )
    x = nc.dram_tensor("x", [rows, F], mybir.dt.float32, kind="ExternalInput")
    out = nc.dram_tensor("out", [rows, NCR], mybir.dt.float32,
                         kind="ExternalOutput")

    in_eng = {"sync": nc.sync, "scalar": nc.scalar, "gpsimd": nc.gpsimd}[in_dma]
    def out_eng_for(t):
        if out_dma == "alt":
            return nc.sync if t % 2 == 0 else nc.scalar
        return {"sync": nc.sync, "scalar": nc.scalar}[out_dma]

    if preload:
        J = rows // P
        x_v = x.rearrange("(p j) c -> p (j c)", p=P)      # [128, J*F]
        out_v = out.rearrange("(p j) c -> p (j c)", p=P)  # [128, J*NCR]
    else:
        x_t = x.rearrange("(t p r) c -> t p (r c)", p=P, r=R)
        out_t = out.rearrange("(t p r) c -> t p (r c)", p=P, r=R)

    def build_tile(t, xt, ot, rr=None):
        rr = R if rr is None else rr
        x3 = xt.rearrange("p (r c) -> p r c", r=rr)
        o3 = ot.rearrange("p (r c) -> p r c", r=rr)
        col = 0
        for i in range(F - 1):
            w = F - 1 - i  # number of pairs starting with feature i
            dst_even = o3[:, :, col:col + 2 * w:2]
            dst_odd = o3[:, :, col + 1:col + 2 * w:2]
            src_b = x3[:, :, i:i + 1].broadcast_to([P, rr, w])
            src_s = x3[:, :, i + 1:F]
            if engines == "split":
                eng = split_pat[i % len(split_pat)]
            else:
                eng = engines[t % len(engines)]
            if eng == "vector":
                nc.vector.tensor_copy(dst_even, src_b)
                nc.vector.tensor_copy(dst_odd, src_s)
            else:
                nc.scalar.copy(dst_even, src_b)
                nc.scalar.copy(dst_odd, src_s)
            col += 2 * w

    def body_ramp(pool, xa, xb):
        # xa holds the first sum(ramp) rows, xb the rest: tile-0 copies
        # only wait on the small head load, so the first store issues
        # ~10us earlier than with one monolithic preload.
        head = sum(ramp)
        in_eng.dma_start(xa[:, :], x_v[:, :head * F])
        in_eng.dma_start(xb[:, :], x_v[:, head * F:])
        j = 0
        for si, rr in enumerate(list(ramp) + [R] * ((rows // P - head) // R)):
            if j < head:
                xt = xa[:, j * F:(j + rr) * F]
            else:
                xt = xb[:, (j - head) * F:(j - head + rr) * F]
            ot = pool.tile([P, R * NCR], mybir.dt.float32, name="ot")
            build_tile(si, xt, ot[:, :rr * NCR], rr=rr)
            out_eng_for(si).dma_start(
                out_v[:, j * NCR:(j + rr) * NCR], ot[:, :rr * NCR])
            j += rr

    def body(pool, xall=None):
        if group > 1:
            # partition-major preload layout: `group` consecutive tiles are
            # contiguous per partition -> one store, half the DMA boundaries
            assert preload and n_tiles % group == 0
            for b in range(n_tiles // group):
                ob = pool.tile([P, group * R * NCR], mybir.dt.float32,
                               name="ob")
                for g in range(group):
                    t = b * group + g
                    xt = xall[:, t * R * F:(t + 1) * R * F]
                    build_tile(t, xt,
                               ob[:, g * R * NCR:(g + 1) * R * NCR])
                out_eng_for(b).dma_start(
                    out_v[:, b * group * R * NCR:
                          (b + 1) * group * R * NCR], ob[:, :])
            return
        for t in range(n_tiles):
            if preload:
                xt = xall[:, t * R * F:(t + 1) * R * F]
            else:
                xtile = pool.tile([P, R * F], mybir.dt.float32, name="xt",
                                  bufs=in_bufs)
                in_eng.dma_start(xtile[:, :], x_t[t])
                xt = xtile[:, :]
            ot = pool.tile([P, R * NCR], mybir.dt.float32, name="ot")
            build_tile(t, xt, ot[:, :])
            if preload:
                out_eng_for(t).dma_start(
                    out_v[:, t * R * NCR:(t + 1) * R * NCR], ot[:, :])
            else:
                out_eng_for(t).dma_start(out_t[t], ot[:, :])

    with TileContext(nc) as tc:
        with tc.tile_pool(name="pool", bufs=bufs) as pool:
            def one_body():
                if ramp:
                    assert preload and sum(ramp) % 2 == 0
                    head = sum(ramp)
                    xa = pool.tile([P, head * F], mybir.dt.float32,
                                   name="xa", bufs=prebufs)
                    xb = pool.tile([P, (rows // P - head) * F],
                                   mybir.dt.float32, name="xb", bufs=prebufs)
                    body_ramp(pool, xa, xb)
                    return
                xall = None
                if preload:
                    xall = pool.tile([P, (rows // P) * F], mybir.dt.float32,
                                     name="xall", bufs=prebufs)
                    in_eng.dma_start(xall[:, :], x_v)
                body(pool, xall)
            if loop:
                with tc.For_i(0, loop, 1):
                    for _ in range(unroll):
                        one_body()
            else:
                for rep in range(repeat):
                    one_body()
    nc.finalize()
    return nc


_NC_CACHE = {}


def _get_nc():
    key = (ROWS_PER_CORE, R_DEFAULT, BUFS_DEFAULT, IN_DMA_DEFAULT,
           ENGINES_DEFAULT, PRELOAD_DEFAULT)
    if key not in _NC_CACHE:
        _NC_CACHE[key] = build_nc()
    return _NC_CACHE[key]


def kernel(x, k=2):
    x = np.ascontiguousarray(np.asarray(x), dtype=np.float32)
    assert int(np.asarray(k)) == 2, "kernel hardcodes k=2"
    B, T, Fin = x.shape
    assert (B, T, Fin) == (B_FULL, T_FULL, F)

    xf = x.reshape(N_CORES, ROWS_PER_CORE, F)
    in_maps = [{"x": xf[c]} for c in range(N_CORES)]
    nc = _get_nc()
    res = run_bass_kernel_spmd(nc, in_maps, list(range(N_CORES)))
    outs = [np.asarray(res.results[c]["out"]) for c in range(N_CORES)]
    return np.concatenate(outs, axis=0).reshape(B, T, NCR)



# revision 7
# speedup vs baseline: 1.0677x; 1.0125x over previous
"""Trainium2 Bass kernel for nn_BuildCombinationsDim2 (k=2 feature-pair gather).

Reference computation: x [B=32, T=4096, F=32] f32, k=2 ->
out[..., j] = x[..., idx[j]] where idx is the flattened list of all
C(32,2)=496 lexicographic feature pairs -> out [32, 4096, 992].

Strategy (pure data movement, memory-bound on the 520MB output write):
  - Shard batch across 8 cores: each core gets [4, 4096, 32] -> 16384 rows.
  - Per core, tile rows onto 128 SBUF partitions, R=8 rows per partition.
  - The 992 output columns decompose into 31 blocks; block i (pairs
    (i,i+1)..(i,31)) has its even slots all equal to x[:,i] (a stride-0
    broadcast copy) and its odd slots equal to the contiguous suffix
    x[:,i+1:32] (a stride-2-dest copy). 62 strided on-chip copies build a
    full [128, R*992] tile, stored as one 4MB contiguous HBM write.
  - Measured floor (this container): store stream ~347 GB/s -> 187us for
    the 62MB/core write; +2MB input load -> ~195us/body. Best measured
    kernel: ~197us/body.
  - fp32 tensor_copy with strided dst / broadcast src runs at 1x mode
    (~1.7 ns/elem/partition), NOT the dense 2x rate: a DVE-only tile build
    is ~13.3us, ABOVE the ~11.1us store cadence -> DVE-only is build-bound
    (~213us). engines="split" assigns block i to DVE (i even) or ACT
    (i odd): ~4us/tile per engine, builds come off the critical path.
  - Input loads go on the ACT HWDGE ring (nc.scalar.dma_start), NOT
    nc.gpsimd/SWDGE (GPSIMD descriptor gen starves behind DVE port locks,
    ~+120us/body) and NOT the sync ring (delays store descriptor gen,
    +10us/body measured at unroll=4).
  - tc.For_i iteration boundaries cost ~4-6us (cross-queue sync); timing
    NEFFs unroll 4 full bodies per iteration to amortize. out_dma="alt",
    group>1, r_per_part 4/16, bufs 2/5/6, prebufs=2 all measured
    neutral-to-worse than sync/R=8/bufs=4.
"""

import numpy as np

import concourse.bacc as bacc
import concourse.bass as bass
import concourse.mybir as mybir
from concourse.bass_utils import run_bass_kernel_spmd
from concourse.tile import TileContext

F = 32
NCR = F * (F - 1)  # 992 = 2 * C(32,2)
N_CORES = 8
P = 128

B_FULL, T_FULL = 32, 4096
ROWS_PER_CORE = (B_FULL // N_CORES) * T_FULL  # 16384

R_DEFAULT = 8        # rows per partition per tile
BUFS_DEFAULT = 4     # output-tile buffering
IN_DMA_DEFAULT = "scalar"  # input DMA on the ACT HWDGE ring (NOT SWDGE)
IN_BUFS_DEFAULT = 8  # input prefetch depth (per-tile load mode only)
ENGINES_DEFAULT = "split"  # co-build every tile on DVE (even blocks) + ACT
                           # (odd blocks); DVE-only builds are build-bound
PRELOAD_DEFAULT = True  # one 2MB input DMA per body: 16 interleaved
                        # per-tile loads cost ~16us/body of store throughput
                        # (HBM read/write turnarounds in the write stream)
RAMP_DEFAULT = (2, 2, 2, 2)  # four 2-row opener tiles fed from a separate
                             # 128KB head-preload tile: first store issues
                             # after ~3us instead of ~14us


def build_nc(rows=ROWS_PER_CORE, r_per_part=R_DEFAULT, bufs=BUFS_DEFAULT,
             engines=ENGINES_DEFAULT, repeat=1, in_dma=IN_DMA_DEFAULT,
             in_bufs=IN_BUFS_DEFAULT, out_dma="sync", loop=0,
             preload=PRELOAD_DEFAULT, unroll=1,
             split_pat=("vector", "scalar"), group=1, ramp=None,
             prebufs=1):
    """Build the per-core Bass module: x [rows, 32] -> out [rows, 992].

    engines: "split" assigns block i of every tile to DVE (i even) or ACT
    (i odd); a tuple like ("vector", "scalar") instead alternates whole
    tiles between engines. repeat>1 unrolls the whole body; loop=N wraps
    the body in a tc.For_i hardware loop (timing harness use).
    """
    if ramp is None:
        # Single-shot: four 2-row opener tiles -> first store after ~2us.
        # Loop mode: ramp=(8,) keeps every store full-size but still splits
        # the preload into a 16KB head (feeds tile 0 after ~1us) + rest,
        # cutting the ~7.5us per-iteration preload serialization (-3us/body
        # measured at unroll=4 vs ramp=()).
        ramp = RAMP_DEFAULT if loop == 0 else (8,)
    if not preload or group > 1:
        ramp = ()
    tile_rows = P * r_per_part
    assert rows % tile_rows == 0
    n_tiles = rows // tile_rows
    R = r_per_part

    nc = bacc.Bacc(
        "TRN2", target_bir_lowering=False, debug=False, num_devices=N_CORES
    )
    x = nc.dram_tensor("x", [rows, F], mybir.dt.float32, kind="ExternalInput")
    out = nc.dram_tensor("out", [rows, NCR], mybir.dt.float32,
                         kind="ExternalOutput")

    in_eng = {"sync": nc.sync, "scalar": nc.scalar, "gpsimd": nc.gpsimd}[in_dma]
    def out_eng_for(t):
        if out_dma == "alt":
            return nc.sync if t % 2 == 0 else nc.scalar
        return {"sync": nc.sync, "scalar": nc.scalar}[out_dma]

    if preload:
        J = rows // P
        x_v = x.rearrange("(p j) c -> p (j c)", p=P)      # [128, J*F]
        out_v = out.rearrange("(p j) c -> p (j c)", p=P)  # [128, J*NCR]
    else:
        x_t = x.rearrange("(t p r) c -> t p (r c)", p=P, r=R)
        out_t = out.rearrange("(t p r) c -> t p (r c)", p=P, r=R)

    def build_tile(t, xt, ot, rr=None):
        rr = R if rr is None else rr
        x3 = xt.rearrange("p (r c) -> p r c", r=rr)
        o3 = ot.rearrange("p (r c) -> p r c", r=rr)
        col = 0
        for i in range(F - 1):
            w = F - 1 - i  # number of pairs starting with feature i
            dst_even = o3[:, :, col:col + 2 * w:2]
            dst_odd = o3[:, :, col + 1:col + 2 * w:2]
            src_b = x3[:, :, i:i + 1].broadcast_to([P, rr, w])
            src_s = x3[:, :, i + 1:F]
            if engines == "split":
                eng = split_pat[i % len(split_pat)]
            else:
                eng = engines[t % len(engines)]
            if eng == "vector":
                nc.vector.tensor_copy(dst_even, src_b)
                nc.vector.tensor_copy(dst_odd, src_s)
            else:
                nc.scalar.copy(dst_even, src_b)
                nc.scalar.copy(dst_odd, src_s)
            col += 2 * w

    def body_ramp(pool, xa, xb):
        # xa holds the first sum(ramp) rows, xb the rest: tile-0 copies
        # only wait on the small head load, so the first store issues
        # ~10us earlier than with one monolithic preload.
        head = sum(ramp)
        in_eng.dma_start(xa[:, :], x_v[:, :head * F])
        in_eng.dma_start(xb[:, :], x_v[:, head * F:])
        j = 0
        for si, rr in enumerate(list(ramp) + [R] * ((rows // P - head) // R)):
            if j < head:
                xt = xa[:, j * F:(j + rr) * F]
            else:
                xt = xb[:, (j - head) * F:(j - head + rr) * F]
            ot = pool.tile([P, R * NCR], mybir.dt.float32, name="ot")
            build_tile(si, xt, ot[:, :rr * NCR], rr=rr)
            out_eng_for(si).dma_start(
                out_v[:, j * NCR:(j + rr) * NCR], ot[:, :rr * NCR])
            j += rr

    def body(pool, xall=None):
        if group > 1:
            # partition-major preload layout: `group` consecutive tiles are
            # contiguous per partition -> one store, half the DMA boundaries
            assert preload and n_tiles % group == 0
            for b in range(n_tiles // group):
                ob = pool.tile([P, group * R * NCR], mybir.dt.float32,
                               name="ob")
                for g in range(group):
                    t = b * group + g
                    xt = xall[:, t * R * F:(t + 1) * R * F]
                    build_tile(t, xt,
                               ob[:, g * R * NCR:(g + 1) * R * NCR])
                out_eng_for(b).dma_start(
                    out_v[:, b * group * R * NCR:
                          (b + 1) * group * R * NCR], ob[:, :])
            return
        for t in range(n_tiles):
            if preload:
                xt = xall[:, t * R * F:(t + 1) * R * F]
            else:
                xtile = pool.tile([P, R * F], mybir.dt.float32, name="xt",
                                  bufs=in_bufs)
                in_eng.dma_start(xtile[:, :], x_t[t])
                xt = xtile[:, :]
            ot = pool.tile([P, R * NCR], mybir.dt.float32, name="ot")
            build_tile(t, xt, ot[:, :])
            if preload:
                out_eng_for(t).dma_start(
                    out_v[:, t * R * NCR:(t + 1) * R * NCR], ot[:, :])
            else:
                out_eng_for(t).dma_start(out_t[t], ot[:, :])

    with TileContext(nc) as tc:
        with tc.tile_pool(name="pool", bufs=bufs) as pool:
            def one_body():
                if ramp:
                    assert preload and sum(ramp) % 2 == 0
                    head = sum(ramp)
                    xa = pool.tile([P, head * F], mybir.dt.float32,
                                   name="xa", bufs=prebufs)
                    xb = pool.tile([P, (rows // P - head) * F],
                                   mybir.dt.float32, name="xb", bufs=prebufs)
                    body_ramp(pool, xa, xb)
                    return
                xall = None
                if preload:
                    xall = pool.tile([P, (rows // P) * F], mybir.dt.float32,
                                     name="xall", bufs=prebufs)
                    in_eng.dma_start(xall[:, :], x_v)
                body(pool, xall)
            if loop:
                with tc.For_i(0, loop, 1):
                    for _ in range(unroll):
                        one_body()
            else:
                for rep in range(repeat):
                    one_body()
    nc.finalize()
    return nc


_NC_CACHE = {}


def _get_nc():
    key = (ROWS_PER_CORE, R_DEFAULT, BUFS_DEFAULT, IN_DMA_DEFAULT,
           ENGINES_DEFAULT, PRELOAD_DEFAULT)
    if key not in _NC_CACHE:
        _NC_CACHE[key] = build_nc()
    return _NC_CACHE[key]


def kernel(x, k=2):
    x = np.ascontiguousarray(np.asarray(x), dtype=np.float32)
    assert int(np.asarray(k)) == 2, "kernel hardcodes k=2"
    B, T, Fin = x.shape
    assert (B, T, Fin) == (B_FULL, T_FULL, F)

    xf = x.reshape(N_CORES, ROWS_PER_CORE, F)
    in_maps = [{"x": xf[c]} for c in range(N_CORES)]
    nc = _get_nc()
    res = run_bass_kernel_spmd(nc, in_maps, list(range(N_CORES)))
    outs = [np.asarray(res.results[c]["out"]) for c in range(N_CORES)]
    return np.concatenate(outs, axis=0).reshape(B, T, NCR)

